# revision 5
# baseline (speedup 1.0000x reference)
"""Trainium2 Bass kernel for the multimodal GRU-D-style LSTM imputation model.

Self-contained: kernel(**inputs) takes the FULL inputs (B=4096) and returns
(loss, sigmoid(y_h), imputations) exactly like the reference.

Strategy: pure data-parallel over 8 NeuronCores (B=512 per core). Each core
runs the full T=128-step scan on its batch shard with bf16 matmuls
(fp32 PSUM accumulation), one activation-table set (exp/tanh/relu) by
expressing sigmoid via tanh, and fp32 state for the LSTM cell c.
Scalar losses are finished on the host from per-core partial sums.

Layout notes (per core, B=512 -> 4 partition chunks of 128):
  - transposed activations live as [feat, batch] tiles: h~=2h bf16 [128,2048]
    fat tiles (col block k = H rows 128k..128k+127), c~=2c fp32 [128,2048].
  - per-step inputs x/m/d loaded natural [128(b), k, t, 96], converted to
    bf16 (f padded to 128 with zeros) and transposed via the DMA xbar.
  - imputations c_c computed in natural layout fp32 (exact where mask=1) and
    DMA'd straight out; the transposed bf16 copy feeds the LSTM gates.
  - sigmoid(z) = (1+tanh(z/2))/2 folded into pre-scaled weights: the kernel
    carries h~=2h and c~=2c; hist_W/Whh/out_W are pre-halved, d3 pre-doubled.
"""

import numpy as np
import ml_dtypes

import concourse.bass as bass
import concourse.bacc as bacc
import concourse.tile as tile
from concourse import mybir

F32 = mybir.dt.float32
BF16 = mybir.dt.bfloat16
AF = mybir.ActivationFunctionType
ALU = mybir.AluOpType
AX = mybir.AxisListType

B_FULL, T_FULL, FD, S, H = 4096, 128, 96, 128, 512
N_CORES = 8
IMPUTE_WEIGHT, L1, L2 = 0.3, 0.01, 0.1


def build_module(Bc=512, T=128, Tc=4):
    """Build the per-core Bass module. Bc = per-core batch, T = steps."""
    NB = Bc // 128          # batch chunks
    NH = H // 128           # hidden chunks (4)
    NG = 4 * H // 128       # gate M-chunks (16)
    assert Bc % 128 == 0 and T % Tc == 0

    nc = bacc.Bacc(target_bir_lowering=False)

    # ---- DRAM I/O ----
    vals = nc.declare_dram_parameter("values", [Bc, T, FD], F32, isOutput=False)
    msks = nc.declare_dram_parameter("masks", [Bc, T, FD], F32, isOutput=False)
    dels = nc.declare_dram_parameter("deltas", [Bc, T, FD], F32, isOutput=False)
    stat = nc.declare_dram_parameter("statics", [Bc, S], F32, isOutput=False)
    smk = nc.declare_dram_parameter("smasks", [Bc, S], F32, isOutput=False)

    w_fr = nc.declare_dram_parameter("w_fr", [S, S], BF16, isOutput=False)        # fr_W.T
    w_d1 = nc.declare_dram_parameter("w_d1", [S, H], BF16, isOutput=False)        # d1_W.T
    w_d2 = nc.declare_dram_parameter("w_d2", [H, H], BF16, isOutput=False)        # d2_W.T
    w_d3 = nc.declare_dram_parameter("w_d3", [H, H], BF16, isOutput=False)        # (2*d3_W).T
    w_td = nc.declare_dram_parameter("w_td", [FD, H], BF16, isOutput=False)       # td_W.T
    w_hist = nc.declare_dram_parameter("w_hist", [H, FD], BF16, isOutput=False)   # (hist_W/2).T
    w_st = nc.declare_dram_parameter("w_st", [S, FD], BF16, isOutput=False)       # st_W.T
    w_real = nc.declare_dram_parameter("w_real", [FD, FD], BF16, isOutput=False)  # real_W.T
    w_icc = nc.declare_dram_parameter("w_icc", [FD, 4 * H], BF16, isOutput=False)  # Wih[:, :96].T
    w_im = nc.declare_dram_parameter("w_im", [FD, 4 * H], BF16, isOutput=False)    # Wih[:, 96:].T
    w_hh = nc.declare_dram_parameter("w_hh", [H, 4 * H], BF16, isOutput=False)     # (Whh/2).T
    w_out = nc.declare_dram_parameter("w_out", [H, 1], BF16, isOutput=False)       # (out_W/2).T

    b_fr = nc.declare_dram_parameter("b_fr", [S, 1], F32, isOutput=False)
    b_d1 = nc.declare_dram_parameter("b_d1", [128, NH], F32, isOutput=False)
    b_d2 = nc.declare_dram_parameter("b_d2", [128, NH], F32, isOutput=False)
    b_d3 = nc.declare_dram_parameter("b_d3", [128, NH], F32, isOutput=False)      # 2*d3_b
    b_ntd = nc.declare_dram_parameter("b_ntd", [128, NH], F32, isOutput=False)    # -td_b
    b_lstm = nc.declare_dram_parameter("b_lstm", [128, NG], F32, isOutput=False)  # bih+bhh
    b_lstmh = nc.declare_dram_parameter("b_lstmh", [128, NG], F32, isOutput=False)  # (bih+bhh)/2
    b_stc = nc.declare_dram_parameter("b_stc", [FD, 1], F32, isOutput=False)      # st_b+real_b+hist_b

    o_imps = nc.declare_dram_parameter("imps", [Bc, T, FD], F32, isOutput=True)
    o_y = nc.declare_dram_parameter("ylog", [1, Bc], F32, isOutput=True)
    o_xnum = nc.declare_dram_parameter("xnum", [128, T], F32, isOutput=True)
    o_xden = nc.declare_dram_parameter("xden", [128, T], F32, isOutput=True)

    vals_v = vals.ap().rearrange("(k p) t f -> p k t f", k=NB)
    msks_v = msks.ap().rearrange("(k p) t f -> p k t f", k=NB)
    dels_v = dels.ap().rearrange("(k p) t f -> p k t f", k=NB)
    imps_v = o_imps.ap().rearrange("(k p) t f -> p k t f", k=NB)
    stat_v = stat.ap().rearrange("(k p) s -> p k s", k=NB)
    smk_v = smk.ap().rearrange("(k p) s -> p k s", k=NB)

    FB = NH * Bc  # fat tile free size (2048 at Bc=512)

    with tile.TileContext(nc) as tc:
        with tc.tile_pool(name="wp", bufs=1) as wp, \
             tc.tile_pool(name="sp1", bufs=1) as sp1, \
             tc.tile_pool(name="sp2", bufs=2) as sp2, \
             tc.tile_pool(name="psG", bufs=4, space="PSUM") as psG, \
             tc.tile_pool(name="psA", bufs=2, space="PSUM") as psA, \
             tc.tile_pool(name="psB", bufs=2, space="PSUM") as psB:

            # ---- persistent loop weights ----
            t_td = wp.tile([FD, H], BF16, name="t_td")
            nc.sync.dma_start(out=t_td, in_=w_td[:, :])
            t_hist = [wp.tile([128, FD], BF16, tag=f"hi_{k}", name=f"hi_{k}") for k in range(NH)]
            for k in range(NH):
                nc.sync.dma_start(out=t_hist[k], in_=w_hist[128 * k:128 * (k + 1), :])
            t_real = wp.tile([FD, FD], BF16, name="t_real")
            nc.sync.dma_start(out=t_real, in_=w_real[:, :])
            t_icc = wp.tile([FD, 4 * H], BF16, name="t_icc")
            nc.sync.dma_start(out=t_icc, in_=w_icc[:, :])
            t_im = wp.tile([FD, 4 * H], BF16, name="t_im")
            nc.sync.dma_start(out=t_im, in_=w_im[:, :])
            t_hh = [wp.tile([128, 4 * H], BF16, tag=f"hh_{k}", name=f"hh_{k}") for k in range(NH)]
            for k in range(NH):
                nc.sync.dma_start(out=t_hh[k], in_=w_hh[128 * k:128 * (k + 1), :])
            t_out = [wp.tile([128, 1], BF16, tag=f"ow_{k}", name=f"ow_{k}") for k in range(NH)]
            for k in range(NH):
                nc.sync.dma_start(out=t_out[k], in_=w_out[128 * k:128 * (k + 1), :])
            t_ntdb = wp.tile([128, NH], F32, name="t_ntdb")
            nc.sync.dma_start(out=t_ntdb, in_=b_ntd[:, :])
            t_lb = wp.tile([128, NG], F32, name="t_lb")
            nc.sync.dma_start(out=t_lb, in_=b_lstm[:, :])
            t_lbh = wp.tile([128, NG], F32, name="t_lbh")
            nc.sync.dma_start(out=t_lbh, in_=b_lstmh[:, :])
            t_stcb = wp.tile([FD, 1], F32, name="t_stcb")
            nc.sync.dma_start(out=t_stcb, in_=b_stc[:, :])
            t_stc = wp.tile([FD, Bc], F32, name="t_stc")

            # persistent accumulators / padded bf16 window tiles
            t_xnum = wp.tile([128, T], F32, name="t_xnum")
            t_xden = wp.tile([128, T], F32, name="t_xden")
            vb = [wp.tile([128, NB, Tc, 128], BF16, tag=f"vb{i}", name=f"vb{i}") for i in range(2)]
            mb = [wp.tile([128, NB, Tc, 128], BF16, tag=f"mb{i}", name=f"mb{i}") for i in range(2)]
            db = [wp.tile([128, NB, Tc, 128], BF16, tag=f"db{i}", name=f"db{i}") for i in range(2)]
            for tl_ in vb + mb + db:
                nc.gpsimd.memset(tl_, 0.0)

            # ---- prologue: statics -> s_cT, h0, st_term (pool freed after) ----
            with tc.tile_pool(name="pre", bufs=1) as pre:
                t_fr = pre.tile([S, S], BF16, name="t_fr")
                nc.sync.dma_start(out=t_fr, in_=w_fr[:, :])
                t_d1 = pre.tile([S, H], BF16, name="t_d1")
                nc.sync.dma_start(out=t_d1, in_=w_d1[:, :])
                t_d2 = [pre.tile([128, H], BF16, tag=f"d2_{k}", name=f"d2_{k}") for k in range(NH)]
                t_d3 = [pre.tile([128, H], BF16, tag=f"d3_{k}", name=f"d3_{k}") for k in range(NH)]
                for k in range(NH):
                    nc.sync.dma_start(out=t_d2[k], in_=w_d2[128 * k:128 * (k + 1), :])
                    nc.sync.dma_start(out=t_d3[k], in_=w_d3[128 * k:128 * (k + 1), :])
                t_frb = pre.tile([S, 1], F32, name="t_frb")
                nc.sync.dma_start(out=t_frb, in_=b_fr[:, :])
                t_d1b = pre.tile([128, NH], F32, name="t_d1b")
                nc.sync.dma_start(out=t_d1b, in_=b_d1[:, :])
                t_d2b = pre.tile([128, NH], F32, name="t_d2b")
                nc.sync.dma_start(out=t_d2b, in_=b_d2[:, :])
                t_d3b = pre.tile([128, NH], F32, name="t_d3b")
                nc.sync.dma_start(out=t_d3b, in_=b_d3[:, :])

                st_nat = pre.tile([128, NB, S], F32, name="st_nat")
                nc.sync.dma_start(out=st_nat, in_=stat_v[:, :, :])
                sm_nat = pre.tile([128, NB, S], F32, name="sm_nat")
                nc.sync.dma_start(out=sm_nat, in_=smk_v[:, :, :])
                st_b16 = pre.tile([128, NB, S], BF16, name="st_b16")
                nc.gpsimd.tensor_copy(st_b16, st_nat)
                sm_b16 = pre.tile([128, NB, S], BF16, name="sm_b16")
                nc.gpsimd.tensor_copy(sm_b16, sm_nat)
                stT = pre.tile([S, Bc], BF16, name="stT")
                smT = pre.tile([S, Bc], BF16, name="smT")
                for k in range(NB):
                    nc.sync.dma_start_transpose(stT[:, 128 * k:128 * (k + 1)], st_b16[:, k, :])
                    nc.sync.dma_start_transpose(smT[:, 128 * k:128 * (k + 1)], sm_b16[:, k, :])
                sh_ps = psG.tile([S, Bc], F32, tag="g", name="sh_ps")
                nc.tensor.matmul(sh_ps, t_fr, stT, start=True, stop=True)
                sh = pre.tile([S, Bc], BF16, name="sh")
                nc.scalar.activation(out=sh, in_=sh_ps, func=AF.Identity, bias=t_frb, scale=1.0)
                sdif = pre.tile([S, Bc], BF16, name="sdif")
                nc.vector.tensor_sub(sdif, stT, sh)
                nc.vector.tensor_mul(sdif, smT, sdif)
                scT = pre.tile([S, Bc], BF16, name="scT")
                nc.vector.tensor_add(scT, sh, sdif)

                # dense chain h1 -> h2 -> h3 (h3 doubled via doubled d3 weights)
                h1 = pre.tile([128, FB], BF16, name="h1")
                for j in range(NH):
                    ps = psG.tile([128, Bc], F32, tag="g", name=f"h1ps{j}")
                    nc.tensor.matmul(ps, t_d1[:, 128 * j:128 * (j + 1)], scT,
                                     start=True, stop=True)
                    nc.scalar.activation(out=h1[:, Bc * j:Bc * (j + 1)], in_=ps, func=AF.Relu,
                                         bias=t_d1b[:, j:j + 1], scale=1.0)
                h2 = pre.tile([128, FB], BF16, name="h2")
                for j in range(NH):
                    ps = psG.tile([128, Bc], F32, tag="g", name=f"h2ps{j}")
                    for k in range(NH):
                        nc.tensor.matmul(ps, t_d2[k][:, 128 * j:128 * (j + 1)],
                                         h1[:, Bc * k:Bc * (k + 1)],
                                         start=(k == 0), stop=(k == NH - 1))
                    nc.scalar.activation(out=h2[:, Bc * j:Bc * (j + 1)], in_=ps, func=AF.Relu,
                                         bias=t_d2b[:, j:j + 1], scale=1.0)
                h_cur = sp2.tile([128, FB], BF16, tag="h", name="h0")
                for j in range(NH):
                    ps = psG.tile([128, Bc], F32, tag="g", name=f"h3ps{j}")
                    for k in range(NH):
                        nc.tensor.matmul(ps, t_d3[k][:, 128 * j:128 * (j + 1)],
                                         h2[:, Bc * k:Bc * (k + 1)],
                                         start=(k == 0), stop=(k == NH - 1))
                    nc.scalar.activation(out=h_cur[:, Bc * j:Bc * (j + 1)], in_=ps, func=AF.Relu,
                                         bias=t_d3b[:, j:j + 1], scale=1.0)

                stc_ps = psB.tile([FD, Bc], F32, tag="xc", name="stc_ps")
                nc.tensor.matmul(stc_ps, t_st_pre(nc, pre, w_st), scT, start=True, stop=True)
                nc.scalar.activation(out=t_stc, in_=stc_ps, func=AF.Identity,
                                     bias=t_stcb, scale=1.0)

            c_cur = sp2.tile([128, FB], F32, tag="c", name="c0")
            nc.vector.memset(c_cur, 0.0)

            GATE_ORDER = list(range(2 * NH, 3 * NH)) + list(range(0, 2 * NH)) + \
                list(range(3 * NH, 4 * NH))

            with tc.tile_pool(name="win", bufs=2) as win, \
                 tc.tile_pool(name="tp", bufs=2) as tp, \
                 tc.tile_pool(name="xp", bufs=2) as xp, \
                 tc.tile_pool(name="np_", bufs=2) as npo:
                vf = mf = df = None
                for t in range(T):
                    w, tl = t // Tc, t % Tc
                    wpi = w % 2
                    if tl == 0:
                        vf = win.tile([128, NB, Tc, FD], F32, tag="vf", name=f"vf{w}")
                        nc.sync.dma_start(out=vf, in_=vals_v[:, :, t:t + Tc, :])
                        mf = win.tile([128, NB, Tc, FD], F32, tag="mf", name=f"mf{w}")
                        nc.sync.dma_start(out=mf, in_=msks_v[:, :, t:t + Tc, :])
                        df = win.tile([128, NB, Tc, FD], F32, tag="df", name=f"df{w}")
                        nc.sync.dma_start(out=df, in_=dels_v[:, :, t:t + Tc, :])
                        nc.gpsimd.tensor_copy(vb[wpi][:, :, :, 0:FD], vf)
                        nc.gpsimd.tensor_copy(mb[wpi][:, :, :, 0:FD], mf)
                        nc.gpsimd.tensor_copy(db[wpi][:, :, :, 0:FD], df)
                        nc.vector.tensor_reduce(
                            out=t_xden[:, t:t + Tc], in_=mf.transpose([0, 2, 1, 3]),
                            axis=AX.XY, op=ALU.add)

                    # transposed step inputs (f on partitions, pad rows >=96 are 0)
                    vT = tp.tile([128, Bc], BF16, tag="vT", name=f"vT{t}")
                    mT = tp.tile([128, Bc], BF16, tag="mT", name=f"mT{t}")
                    dT = tp.tile([128, Bc], BF16, tag="dT", name=f"dT{t}")
                    for k in range(NB):
                        nc.sync.dma_start_transpose(vT[:, 128 * k:128 * (k + 1)],
                                                    vb[wpi][:, k, tl, :])
                        nc.sync.dma_start_transpose(mT[:, 128 * k:128 * (k + 1)],
                                                    mb[wpi][:, k, tl, :])
                        nc.sync.dma_start_transpose(dT[:, 128 * k:128 * (k + 1)],
                                                    db[wpi][:, k, tl, :])

                    # gamma = min(exp(-(z + td_b)), 1), z = td_W @ dT; then *= 2h
                    gam = sp1.tile([128, FB], BF16, tag="gam", name=f"gam{t}")
                    for j in range(NH):
                        ps = psA.tile([128, Bc], F32, tag="gps", name=f"gps{t}_{j}")
                        nc.tensor.matmul(ps, t_td[:, 128 * j:128 * (j + 1)], dT[0:FD, :],
                                         start=True, stop=True)
                        nc.scalar.activation(out=gam[:, Bc * j:Bc * (j + 1)], in_=ps,
                                             func=AF.Exp, bias=t_ntdb[:, j:j + 1], scale=-1.0)
                    nc.vector.tensor_scalar_min(gam, gam, 1.0)
                    nc.vector.tensor_mul(gam, h_cur, gam)
                    hp = gam  # decayed hidden 2h*gamma, in place

                    # x_cT = real_W @ xT + (hist_W/2) @ hp + st_term(+biases)
                    xps = psB.tile([FD, Bc], F32, tag="xc", name=f"xc{t}")
                    nc.tensor.matmul(xps, t_real, vT[0:FD, :], start=True, stop=False)
                    for k in range(NH):
                        nc.tensor.matmul(xps, t_hist[k], hp[:, Bc * k:Bc * (k + 1)],
                                         start=False, stop=(k == NH - 1))
                    xcb = xp.tile([FD, Bc], BF16, tag="xcb", name=f"xcb{t}")
                    nc.vector.tensor_add(xcb, xps, t_stc)

                    # transposed blend for the gate input c_cT
                    difT = xp.tile([FD, Bc], BF16, tag="difT", name=f"difT{t}")
                    nc.vector.tensor_sub(difT, vT[0:FD, :], xcb)
                    nc.vector.tensor_mul(difT, mT[0:FD, :], difT)
                    ccT = xp.tile([FD, Bc], BF16, tag="ccT", name=f"ccT{t}")
                    nc.vector.tensor_add(ccT, xcb, difT)

                    # natural-layout blend: exact output + loss pieces
                    xcn = npo.tile([128, NB, FD], BF16, tag="xcn", name=f"xcn{t}")
                    for k in range(NB):
                        nc.sync.dma_start_transpose(xcn[:, k, :], xcb[:, 128 * k:128 * (k + 1)])
                    xcnf = npo.tile([128, NB, FD], F32, tag="xcnf", name=f"xcnf{t}")
                    nc.vector.tensor_copy(xcnf, xcn)
                    dif = npo.tile([128, NB, FD], F32, tag="dif", name=f"dif{t}")
                    nc.vector.tensor_sub(dif, vf[:, :, tl, :], xcnf)
                    nc.vector.tensor_mul(dif, mf[:, :, tl, :], dif)
                    cc = npo.tile([128, NB, FD], F32, tag="cc", name=f"cc{t}")
                    nc.vector.tensor_add(cc, xcnf, dif)
                    nc.vector.tensor_reduce(out=t_xnum[:, t:t + 1], in_=dif, axis=AX.XY,
                                            op=ALU.add, apply_absolute_value=True)
                    nc.sync.dma_start(out=imps_v[:, :, t, :], in_=cc)

                    # LSTM gates: tau = tanh((z+b)/2) for i,f,o ; g = tanh(z+b)
                    tau_i = sp1.tile([128, FB], BF16, tag="ti", name=f"ti{t}")
                    tau_f = sp1.tile([128, FB], F32, tag="tf", name=f"tf{t}")
                    g_t = sp1.tile([128, FB], BF16, tag="tg", name=f"tg{t}")
                    tau_o = sp1.tile([128, FB], BF16, tag="to", name=f"to{t}")
                    targets = [(tau_i, 0.5, t_lbh), (tau_f, 0.5, t_lbh),
                               (g_t, 1.0, t_lb), (tau_o, 0.5, t_lbh)]
                    for j in GATE_ORDER:
                        gate, col = j // NH, j % NH
                        tgt, scl, bias = targets[gate]
                        ps = psG.tile([128, Bc], F32, tag="g", name=f"g{t}_{j}")
                        nc.tensor.matmul(ps, t_im[:, 128 * j:128 * (j + 1)], mT[0:FD, :],
                                         start=True, stop=False)
                        nc.tensor.matmul(ps, t_icc[:, 128 * j:128 * (j + 1)], ccT,
                                         start=False, stop=False)
                        for k in range(NH):
                            nc.tensor.matmul(ps, t_hh[k][:, 128 * j:128 * (j + 1)],
                                             hp[:, Bc * k:Bc * (k + 1)],
                                             start=False, stop=(k == NH - 1))
                        nc.scalar.activation(out=tgt[:, Bc * col:Bc * (col + 1)], in_=ps,
                                             func=AF.Tanh, bias=bias[:, j:j + 1], scale=scl)

                    # c~_new = 0.5*(c~ + tau_f*c~) + (g + tau_i*g)   [in-place reuse]
                    nc.vector.tensor_mul(tau_i, tau_i, g_t)            # u = ti*g
                    nc.vector.tensor_add(g_t, g_t, tau_i)              # s2 = g + u
                    nc.gpsimd.tensor_mul(tau_f, tau_f, c_cur)          # v = tf*c
                    nc.vector.tensor_add(c_cur, c_cur, tau_f)          # s1 = c + v
                    c_new = sp2.tile([128, FB], F32, tag="c", name=f"c{t}")
                    nc.vector.scalar_tensor_tensor(c_new, c_cur, 0.5, g_t, ALU.mult, ALU.add)
                    # h~_new = tau_c + tau_o*tau_c,  tau_c = tanh(c~/2)
                    tau_c = sp1.tile([128, FB], BF16, tag="tc", name=f"tc{t}")
                    nc.scalar.activation(out=tau_c, in_=c_new, func=AF.Tanh, scale=0.5)
                    nc.vector.tensor_mul(tau_o, tau_o, tau_c)          # w = to*tc
                    h_new = sp2.tile([128, FB], BF16, tag="h", name=f"h{t}")
                    nc.vector.tensor_add(h_new, tau_c, tau_o)
                    h_cur, c_cur = h_new, c_new

                # ---- epilogue: y = (out_W/2) @ h~ ----
                yps = psB.tile([1, Bc], F32, tag="xc", name="yps")
                for k in range(NH):
                    nc.tensor.matmul(yps, t_out[k], h_cur[:, Bc * k:Bc * (k + 1)],
                                     start=(k == 0), stop=(k == NH - 1))
                t_y = win.tile([1, Bc], F32, tag="ty", name="t_y")
                nc.vector.tensor_copy(t_y, yps)
                nc.sync.dma_start(out=o_y[:, :], in_=t_y)
                nc.sync.dma_start(out=o_xnum[:, :], in_=t_xnum)
                nc.sync.dma_start(out=o_xden[:, :], in_=t_xden)

    return nc


def t_st_pre(nc, pre, w_st):
    t_st = pre.tile([S, FD], BF16, name="t_st")
    nc.sync.dma_start(out=t_st, in_=w_st[:, :])
    return t_st


def prep_weights(inputs, Bc=512):
    """Host-side weight repack (bf16 casts, transposes, folding)."""
    bf = ml_dtypes.bfloat16
    f32 = np.float32
    NH, NG = H // 128, 4 * H // 128

    def colmaj(b, n):  # [n*128] -> [128, n] with col j = b[128j:128j+128]
        return np.ascontiguousarray(np.asarray(b, f32).reshape(n, 128).T)

    w = {}
    w["w_fr"] = np.ascontiguousarray(np.asarray(inputs["fr_W"], f32).T).astype(bf)
    w["w_d1"] = np.ascontiguousarray(np.asarray(inputs["d1_W"], f32).T).astype(bf)
    w["w_d2"] = np.ascontiguousarray(np.asarray(inputs["d2_W"], f32).T).astype(bf)
    w["w_d3"] = np.ascontiguousarray(2.0 * np.asarray(inputs["d3_W"], f32).T).astype(bf)
    w["w_td"] = np.ascontiguousarray(np.asarray(inputs["td_W"], f32).T).astype(bf)
    w["w_hist"] = np.ascontiguousarray(0.5 * np.asarray(inputs["hist_W"], f32).T).astype(bf)
    w["w_st"] = np.ascontiguousarray(np.asarray(inputs["st_W"], f32).T).astype(bf)
    w["w_real"] = np.ascontiguousarray(np.asarray(inputs["real_W"], f32).T).astype(bf)
    wih = np.asarray(inputs["lstm_Wih"], f32)
    w["w_icc"] = np.ascontiguousarray(wih[:, :FD].T).astype(bf)
    w["w_im"] = np.ascontiguousarray(wih[:, FD:].T).astype(bf)
    w["w_hh"] = np.ascontiguousarray(0.5 * np.asarray(inputs["lstm_Whh"], f32).T).astype(bf)
    w["w_out"] = np.ascontiguousarray(0.5 * np.asarray(inputs["out_W"], f32).T).astype(bf)

    w["b_fr"] = np.asarray(inputs["fr_b"], f32).reshape(S, 1)
    w["b_d1"] = colmaj(inputs["d1_b"], NH)
    w["b_d2"] = colmaj(inputs["d2_b"], NH)
    w["b_d3"] = colmaj(2.0 * np.asarray(inputs["d3_b"], f32), NH)
    w["b_ntd"] = colmaj(-np.asarray(inputs["td_b"], f32), NH)
    lb = np.asarray(inputs["lstm_bih"], f32) + np.asarray(inputs["lstm_bhh"], f32)
    w["b_lstm"] = colmaj(lb, NG)
    w["b_lstmh"] = colmaj(0.5 * lb, NG)
    w["b_stc"] = (np.asarray(inputs["st_b"], f32) + np.asarray(inputs["real_b"], f32)
                  + np.asarray(inputs["hist_b"], f32)).reshape(FD, 1)
    return w


_NC_CACHE = {}


def kernel(**inputs):
    from concourse.bass_utils import run_bass_kernel_spmd

    Bc = B_FULL // N_CORES
    key = (Bc, T_FULL)
    if key not in _NC_CACHE:
        nc_ = build_module(Bc=Bc, T=T_FULL, Tc=4)
        nc_.compile()
        _NC_CACHE[key] = nc_
    nc = _NC_CACHE[key]

    f32 = np.float32
    vals = np.ascontiguousarray(np.asarray(inputs["values"], f32))
    msks = np.ascontiguousarray(np.asarray(inputs["masks"], f32))
    dels = np.ascontiguousarray(np.asarray(inputs["deltas"], f32))
    stat = np.ascontiguousarray(np.asarray(inputs["statics"], f32))
    smk = np.ascontiguousarray(np.asarray(inputs["static_masks"], f32))
    labels = np.asarray(inputs["labels"], f32)

    w = prep_weights(inputs, Bc)
    in_maps = []
    for c in range(N_CORES):
        sl = slice(c * Bc, (c + 1) * Bc)
        m = dict(values=vals[sl], masks=msks[sl], deltas=dels[sl],
                 statics=stat[sl], smasks=smk[sl])
        m.update(w)
        in_maps.append(m)

    res = run_bass_kernel_spmd(nc, in_maps, core_ids=list(range(N_CORES)))
    outs = res.results
    return finish_outputs(inputs, outs)


def finish_outputs(inputs, outs):
    f32 = np.float32
    labels = np.asarray(inputs["labels"], f32)
    imputations = np.concatenate([o["imps"] for o in outs], axis=0)
    z = np.concatenate([o["ylog"][0] for o in outs]) + f32(np.asarray(inputs["out_b"], f32)[0])
    xnum = np.stack([o["xnum"] for o in outs])  # [cores, 128, T]
    xden = np.stack([o["xden"] for o in outs])

    num_t = xnum.sum(axis=(0, 1), dtype=np.float64)
    den_t = xden.sum(axis=(0, 1), dtype=np.float64)
    x_loss = float((num_t / (den_t + 1e-5)).sum())

    z = z.astype(f32)
    y_loss = float(np.mean(np.maximum(z, 0.0) - z * labels + np.log1p(np.exp(-np.abs(z)))))

    real_W = np.asarray(inputs["real_W"], f32)
    hist_W = np.asarray(inputs["hist_W"], f32)
    st_W = np.asarray(inputs["st_W"], f32)
    fr_W = np.asarray(inputs["fr_W"], f32)
    r_loss = L1 * (np.abs(real_W).sum() + np.abs(hist_W).sum() + np.abs(st_W).sum()) \
        + L2 * np.abs(np.diagonal(real_W)).sum()
    s_loss = IMPUTE_WEIGHT * L2 * np.abs(np.diagonal(fr_W)).sum()

    loss = np.float32(y_loss + float(r_loss) + IMPUTE_WEIGHT * x_loss + float(s_loss))
    y_prob = (1.0 / (1.0 + np.exp(-z))).astype(f32).reshape(-1, 1)
    return (loss, y_prob, imputations.astype(f32))
